# revision 4
# baseline (speedup 1.0000x reference)
"""BertAttention (cross-attention, eval) on 8 Trainium2 NeuronCores — v3.

Same math/sharding as the original baseline (4 batches x 2 head-groups,
er-trick softmax with the denominator row riding in the PV matmul), with
the epilogue and schedule restructured after HW microbenchmarking showed
the steady state was stall-bound (ACT exp floor ~1.1us/step; PE matmuls
stream bf16 at 2 elem/cycle so PE has large slack):

  * The kernel emits RAW context accumulators: for each (pair, head)
    a [65, 512] block = 64 unnormalized context rows + the softmax
    denominator row.  out dram is [pairs*2*65 = 520, sq].
  * Normalization (divide by denominator) and the +bv bias happen on the
    HOST after the gather.  This removes the reciprocal + partition
    broadcast (Pool) + multiply + bias-add chain per (pair, qb, head) —
    measured as the largest single stall (~+600ns/step in an isolated
    A/B microbench) — and releases ctx PSUM banks with one DVE copy.
  * vproj's er-scaling is one tensor_tensor per k-chunk (er broadcast
    along the head dim via a stride-0 AP) instead of 8 tensor_scalar_mul.
  * Per step the previous step's PV matmuls are emitted into the PE FIFO
    BEFORE any woven projection work (their pt input is already ready,
    so they never head-of-line-block the next step's score matmuls),
    and the pt pool is deepened to 8 so ACT can run ahead of PV.

Measured steady-state body time (pipelined async differential, 8 cores):
baseline 445us -> v3 ~367us per body.
"""

import numpy as np
import ml_dtypes

import concourse.mybir as mybir
import concourse.tile as tile
from concourse import bacc
from concourse.bass_utils import run_bass_kernel_spmd

P = 128
B, SQ, SK, HID, NH = 4, 2048, 2048, 1024, 16
HD = 64
N_CORES = 8
NHC = NH // 2          # heads per core = 8
DW = NHC * HD          # per-core projection width = 512
OW = NHC * (HD + 1)    # out rows per core = 520

_BF = ml_dtypes.bfloat16


def build_nc(sq=SQ, sk=SK, hid=HID, nhc=NHC, reps=1):
    hd = HD
    cc_n = hid // P          # contraction chunks (8)
    kc_n = sk // P           # key chunks (16)
    pairs = nhc // 2         # 4
    dw = nhc * hd            # 512
    vw = nhc * (hd + 1)      # 520
    qb_n = sq // 512         # q blocks (4)
    kg_n = sk // 512         # K-proj groups (4)
    qg_n = sq // 512         # Q-proj groups (4)

    bf = mybir.dt.bfloat16
    f32 = mybir.dt.float32
    Exp = mybir.ActivationFunctionType.Exp
    MULT = mybir.AluOpType.mult

    nc = bacc.Bacc("TRN2", target_bir_lowering=False, debug=False)

    xT = nc.dram_tensor("xT", [hid, sq], bf, kind="ExternalInput").ap()
    cT = nc.dram_tensor("cT", [hid, sk], bf, kind="ExternalInput").ap()
    wq = nc.dram_tensor("wq", [hid, dw], bf, kind="ExternalInput").ap()
    wk = nc.dram_tensor("wk", [hid, dw], bf, kind="ExternalInput").ap()
    wv = nc.dram_tensor("wv", [hid, dw], bf, kind="ExternalInput").ap()
    er = nc.dram_tensor("er", [P, kc_n * nhc], f32, kind="ExternalInput").ap()
    out = nc.dram_tensor("out", [pairs * 2 * (hd + 1), sq], f32,
                         kind="ExternalOutput").ap()

    with tile.TileContext(nc) as tc:
        with (
            tc.tile_pool(name="in2", bufs=2) as ipool,     # rep-overlapped
            tc.tile_pool(name="in1", bufs=1) as xpool,     # frees mid-rep
            tc.tile_pool(name="qk", bufs=2) as qkpool,
            tc.tile_pool(name="pt", bufs=8) as ptpool,
            tc.tile_pool(name="work", bufs=2) as wpool,
            tc.tile_pool(name="psum", bufs=2, space="PSUM") as pspool,
        ):
            def alloc_tiles():
                t = {}
                t["xT_sb"] = xpool.tile([P, cc_n * sq], bf, name="xT_sb")
                t["cT_sb"] = ipool.tile([P, cc_n * sk], bf, name="cT_sb")
                t["wq_sb"] = xpool.tile([P, cc_n * dw], bf, name="wq_sb")
                t["wk_sb"] = xpool.tile([P, cc_n * dw], bf, name="wk_sb")
                t["wv_sb"] = xpool.tile([P, cc_n * dw], bf, name="wv_sb")
                t["v_sb"] = ipool.tile([P, kc_n * vw], bf, name="v_sb")
                t["er_sb"] = xpool.tile([P, kc_n * nhc], f32, name="er_sb")
                t["qkt"] = {}
                t["proj_ps"] = {}
                return t

            def emit_dmas(t):
                cT_d = t["cT_sb"].rearrange("p (c s) -> p c s", c=cc_n)
                cT_s = cT.rearrange("(c p) s -> p c s", p=P)
                xT_d = t["xT_sb"].rearrange("p (c s) -> p c s", c=cc_n)
                xT_s = xT.rearrange("(c p) s -> p c s", p=P)

                def col_group(dst, src, g):
                    nc.sync.dma_start(dst[:, :, g * 512:(g + 1) * 512],
                                      src[:, :, g * 512:(g + 1) * 512])

                nc.sync.dma_start(
                    t["wv_sb"].rearrange("p (c w) -> p c w", c=cc_n),
                    wv.rearrange("(c p) w -> p c w", p=P))
                col_group(cT_d, cT_s, 0)
                nc.sync.dma_start(t["er_sb"][:, :], er[:, :])
                nc.sync.dma_start(
                    t["wk_sb"].rearrange("p (c w) -> p c w", c=cc_n),
                    wk.rearrange("(c p) w -> p c w", p=P))
                col_group(cT_d, cT_s, 1)
                nc.sync.dma_start(
                    t["wq_sb"].rearrange("p (c w) -> p c w", c=cc_n),
                    wq.rearrange("(c p) w -> p c w", p=P))
                col_group(xT_d, xT_s, 0)
                col_group(cT_d, cT_s, 2)
                col_group(cT_d, cT_s, 3)
                for g in range(1, 4):
                    col_group(xT_d, xT_s, g)

            def emit_vproj(t, kc):
                pv_ps = pspool.tile([P, 512], f32, tag="pj", name="pv_ps")
                for cc in range(cc_n):
                    nc.tensor.matmul(
                        pv_ps[:, :],
                        lhsT=t["cT_sb"][:, cc * sk + kc * P: cc * sk + kc * P + P],
                        rhs=t["wv_sb"][:, cc * dw:(cc + 1) * dw],
                        start=(cc == 0), stop=(cc == cc_n - 1))
                base = kc * vw
                v_sb, er_sb = t["v_sb"], t["er_sb"]
                # er-scale all 8 heads in one tensor_tensor: er column
                # broadcast along the head dim via a stride-0 inner dim.
                vdst = v_sb[:, base:base + vw].rearrange(
                    "p (h w) -> p h w", h=nhc)[:, :, 0:hd]
                pv3 = pv_ps.rearrange("p (h d) -> p h d", h=nhc)
                er3 = er_sb[:, kc * nhc:(kc + 1) * nhc].rearrange(
                    "p (h o) -> p h o", o=1).broadcast_to((P, nhc, hd))
                nc.vector.tensor_tensor(vdst, pv3, er3, MULT)
                edst = v_sb[:, base:base + vw].rearrange(
                    "p (h w) -> p h w", h=nhc)[:, :, hd:hd + 1]
                nc.vector.tensor_copy(
                    edst, er_sb[:, kc * nhc:(kc + 1) * nhc].rearrange(
                        "p (h w) -> p h w", w=1))

            def get_qkt(t, pp):
                if pp not in t["qkt"]:
                    qt = qkpool.tile([P, sq], bf, tag="qt", name=f"qt{pp}")
                    kt = qkpool.tile([P, sk], bf, tag="kt", name=f"kt{pp}")
                    t["qkt"][pp] = (qt, kt)
                return t["qkt"][pp]

            def emit_proj_part(t, kind, pp, g, c0, c1):
                key = (kind, pp, g)
                if key not in t["proj_ps"]:
                    t["proj_ps"][key] = pspool.tile([P, 512], f32, tag="pj",
                                                    name=f"{kind}_ps")
                ps = t["proj_ps"][key]
                w_sb = t["wk_sb"] if kind == "k" else t["wq_sb"]
                src = t["cT_sb"] if kind == "k" else t["xT_sb"]
                for cc in range(c0, c1):
                    nc.tensor.matmul(
                        ps[:, :],
                        lhsT=w_sb[:, cc * dw + pp * P: cc * dw + pp * P + P],
                        rhs=src[:, cc * sk + g * 512: cc * sk + (g + 1) * 512],
                        start=(cc == 0), stop=(cc == cc_n - 1))
                if c1 == cc_n:
                    qt, kt = get_qkt(t, pp)
                    dst = kt if kind == "k" else qt
                    nc.vector.tensor_copy(dst[:, g * 512:(g + 1) * 512],
                                          ps[:, :])
                    del t["proj_ps"][key]

            def prologue_units(t, n_vproj):
                units = [(lambda k: lambda: emit_vproj(t, k))(kc)
                         for kc in range(n_vproj)]
                for c0 in range(0, cc_n, 2):
                    units.append((lambda c: lambda:
                                  emit_proj_part(t, "k", 0, 0, c, c + 2))(c0))
                for c0 in range(0, cc_n, 2):
                    units.append((lambda c: lambda:
                                  emit_proj_part(t, "q", 0, 0, c, c + 2))(c0))
                return units

            def emit_out(state):
                """Copy raw ctx (64 rows + denom row) to SBUF, DMA to DRAM.
                Normalization happens on the host."""
                p, qb = state["p"], state["qb"]
                for hh, ctx_ps in ((0, state["ctx0"]), (1, state["ctx1"])):
                    o_sb = wpool.tile([hd + 1, 512], f32, tag="osb",
                                      name="o_sb", bufs=6)
                    nc.vector.tensor_copy(o_sb[:, :], ctx_ps[:, :])
                    row0 = (p * 2 + hh) * (hd + 1)
                    nc.sync.dma_start(
                        out[row0: row0 + hd + 1,
                            qb * 512:(qb + 1) * 512],
                        o_sb[:, :])

            def emit_pv(state, kc, pt):
                p, v_sb = state["p"], state["t"]["v_sb"]
                h0, h1 = 2 * p, 2 * p + 1
                nc.tensor.matmul(
                    state["ctx0"][:, :],
                    lhsT=v_sb[:, kc * vw + h0 * (hd + 1):
                              kc * vw + (h0 + 1) * (hd + 1)],
                    rhs=pt[:, 0:512],
                    start=(kc == 0), stop=(kc == kc_n - 1))
                nc.tensor.matmul(
                    state["ctx1"][:, :],
                    lhsT=v_sb[:, kc * vw + h1 * (hd + 1):
                              kc * vw + (h1 + 1) * (hd + 1)],
                    rhs=pt[:, 512:1024],
                    start=(kc == 0), stop=(kc == kc_n - 1))

            def emit_body(t, next_t, chase_from, carry_over):
                sched = {}

                def add(p, qb, kc, fn):
                    sched.setdefault((p, qb, kc), []).append(fn)

                def add_split(p, qb, kc0, kind, pp, g):
                    for u in range(4):
                        add(p, qb, kc0 + u,
                            (lambda kd, ppp, gg, c0: lambda:
                             emit_proj_part(t, kd, ppp, gg, c0, c0 + 2))(
                                 kind, pp, g, 2 * u))

                for kc in range(chase_from, kc_n):
                    add(0, 0, kc - (chase_from - 1),
                        (lambda k: lambda: emit_vproj(t, k))(kc))
                add(0, 0, 2, lambda: emit_proj_part(t, "k", 0, 1, 0, cc_n))
                add(0, 0, 6, lambda: emit_proj_part(t, "k", 0, 2, 0, cc_n))
                add(0, 0, 10, lambda: emit_proj_part(t, "k", 0, 3, 0, cc_n))
                add(0, 0, 14, lambda: emit_proj_part(t, "q", 0, 1, 0, cc_n))
                add_split(0, 1, 0, "q", 0, 2)
                add_split(0, 1, 4, "q", 0, 3)
                for p in range(pairs - 1):
                    for g in range(kg_n):
                        add_split(p, 2, 4 * g, "k", p + 1, g)
                    for g in range(qg_n):
                        add_split(p, 3, 4 * g, "q", p + 1, g)
                for u, fn in enumerate(carry_over):
                    qb, kc = 2 + u // 8, 2 * (u % 8) + 1
                    add(3, qb, kc, fn)

                prev = None
                state = None
                for i in range(pairs * qb_n * kc_n):
                    p, r = divmod(i, qb_n * kc_n)
                    qb, kc = divmod(r, kc_n)
                    if next_t is not None and p == 3 and qb == 0 and kc == 0:
                        emit_dmas(next_t)
                    qt_sb, kt_sb = get_qkt(t, p)
                    if kc == 0:
                        t["qkt"].pop(p - 1, None)
                        ctx0 = pspool.tile([hd + 1, 512], f32, tag="ctx",
                                           name="ctx0")
                        ctx1 = pspool.tile([hd + 1, 512], f32, tag="ctx",
                                           name="ctx1")
                        state = {"p": p, "qb": qb, "ctx0": ctx0, "ctx1": ctx1,
                                 "t": t}
                    qs = qb * 512
                    st = pspool.tile([P, 1024], f32, tag="st", name="st")
                    nc.tensor.matmul(
                        st[:, 0:512],
                        lhsT=kt_sb[0:64, kc * P:(kc + 1) * P],
                        rhs=qt_sb[0:64, qs:qs + 512],
                        start=True, stop=True, tile_position=(0, 0))
                    nc.tensor.matmul(
                        st[:, 512:1024],
                        lhsT=kt_sb[64:128, kc * P:(kc + 1) * P],
                        rhs=qt_sb[64:128, qs:qs + 512],
                        start=True, stop=True, tile_position=(64, 0))
                    pt = ptpool.tile([P, 1024], bf, tag="pt", name="pt")
                    nc.scalar.activation(pt, st, Exp)
                    # PV of the previous step goes into the PE FIFO before
                    # any proj work: its input (prev pt) is already ready,
                    # so it never head-of-line-blocks the next scores.
                    if prev is not None:
                        pstate = prev[0]
                        emit_pv(*prev)
                        if prev[1] == kc_n - 1:
                            emit_out(pstate)
                    for fn in sched.pop((p, qb, kc), ()):
                        fn()
                    prev = (state, kc, pt)
                emit_pv(*prev)
                emit_out(prev[0])
                assert not sched, f"unscheduled work: {list(sched)}"

            t = alloc_tiles()
            emit_dmas(t)
            for fn in prologue_units(t, 4):
                fn()
            chase = 4
            for r in range(reps):
                next_t = alloc_tiles() if r + 1 < reps else None
                carry = prologue_units(next_t, 8) if next_t else []
                emit_body(t, next_t, chase, carry)
                t = next_t
                chase = 8

    nc.compile()
    return nc


_NC_CACHE = {}


def _get_nc():
    if "nc" not in _NC_CACHE:
        _NC_CACHE["nc"] = build_nc()
    return _NC_CACHE["nc"]


def _prep_core_inputs(hidden_states, context, Wq, bq, Wk, bk, Wv, bv):
    scale = 1.0 / np.sqrt(HD)
    xT_b, cT_b = [], []
    for b in range(B):
        xT_b.append(np.ascontiguousarray(hidden_states[b].T).astype(_BF))
        cT_b.append(np.ascontiguousarray(context[b].T).astype(_BF))
    in_maps = []
    for c in range(N_CORES):
        b = c // 2
        hs = (c % 2) * NHC
        cols = slice(hs * HD, (hs + NHC) * HD)
        wq_c = (Wq[:, cols] * scale).astype(_BF)
        wk_c = Wk[:, cols].astype(_BF)
        wv_c = Wv[:, cols].astype(_BF)
        wkr = np.empty((HID, NHC), np.float32)
        for h in range(NHC):
            hcols = slice((hs + h) * HD, (hs + h + 1) * HD)
            wkr[:, h] = (Wk[:, hcols] @ bq[hcols]) * scale
        rT = np.asarray(context[b], np.float32) @ wkr        # [SK, NHC]
        er_c = np.exp(rT).reshape(SK // P, P, NHC).transpose(1, 0, 2)
        er_c = np.ascontiguousarray(er_c.reshape(P, -1), np.float32)
        in_maps.append({
            "xT": xT_b[b],
            "cT": cT_b[b],
            "wq": np.ascontiguousarray(wq_c),
            "wk": np.ascontiguousarray(wk_c),
            "wv": np.ascontiguousarray(wv_c),
            "er": er_c,
        })
    return in_maps


def kernel(hidden_states, context, Wq, bq, Wk, bk, Wv, bv):
    hidden_states = np.asarray(hidden_states, dtype=np.float32)
    context = np.asarray(context, dtype=np.float32)
    Wq = np.asarray(Wq, dtype=np.float32)
    bq = np.asarray(bq, dtype=np.float32)
    Wk = np.asarray(Wk, dtype=np.float32)
    bk = np.asarray(bk, dtype=np.float32)
    Wv = np.asarray(Wv, dtype=np.float32)
    bv = np.asarray(bv, dtype=np.float32)

    nc = _get_nc()
    in_maps = _prep_core_inputs(hidden_states, context, Wq, bq, Wk, bk, Wv, bv)
    res = run_bass_kernel_spmd(nc, in_maps, list(range(N_CORES)))
    full = np.empty((B, SQ, NH * HD), np.float32)
    for c in range(N_CORES):
        b = c // 2
        hs = (c % 2) * NHC
        raw = res.results[c]["out"]               # [520, SQ]
        raw3 = raw.reshape(NHC, HD + 1, SQ)
        ctx = raw3[:, :HD, :] / raw3[:, HD:HD + 1, :]   # [NHC, HD, SQ]
        for h in range(NHC):
            cols = slice((hs + h) * HD, (hs + h + 1) * HD)
            full[b, :, cols] = ctx[h].T + bv[cols]
    return full
